# revision 23
# baseline (speedup 1.0000x reference)
"""YOLOv3 detection-layer kernel for Trainium2 (Bass/Tile), 8-core data parallel.

Math (per image, input x [255, 5776] channel-major, f = a*85 + c):
  out_flat[hw, f] = g_f(x[f, hw])   where out_flat is [5776, 255] and the
  full output [17328, 85] is just out_flat reshaped (box = hw*3 + a).
So the kernel is: DMA load (channels on partitions) -> PE transpose-mode
(128x128 tiles, exact routing) into PSUM [hw, 255] -> fused sigmoid +
grid/anchor affine -> contiguous DMA store.

Per anchor a (cols base = 85*a), with s = sigmoid(x) over ALL 255 attrs
(one activation instruction per group; exp comes from the sigmoid identity
exp(z) = s/(1-s), with the anchor scale folded into the reciprocal):
  t1 = (s_wh - 1) * (-1/av)   # = (1-s)/av,  av = anchor_wh/(2*608)
  t1 = 1/t1                   # = av/(1-s)
  t1 = t1 * s_wh              # = exp(wh)*av = half
  t2 = s_xy * (1.05/76) + (g-0.025)/76   # = imxy
  out[0:2] = t2 - t1 ; out[2:4] = t2 + t1 ; out[4:85] = s (already there)
"""

import os

import numpy as np

import concourse.bacc as bacc
import concourse.mybir as mybir
import concourse.tile as tile
from concourse.alu_op_type import AluOpType
from concourse.bass_utils import run_bass_kernel_spmd
from concourse.masks import make_identity

F32 = mybir.dt.float32

B = 32            # batch
NCH = 255         # channels = 3 anchors * 85 attrs
H = W = 76
HW = H * W        # 5776
NCORES = 8
IPC = B // NCORES  # images per core
XY_SCALE = 1.05
KSC = XY_SCALE / W
ANCHOR_WH = [(10.0, 13.0), (16.0, 30.0), (33.0, 23.0)]

# Each group owns 4 PSUM banks and covers 512 (tail: 144) consecutive output
# rows. Within a group, PSUM partition p of bank t holds output row
# base + 4p + t, so each partition stores ONE contiguous 4080B DRAM chunk
# (4 adjacent 1020B rows) -> 128 descriptors per store instead of 512.
# (group_index, partitions) ; group 11 is the 144-row tail (36 partitions).
GROUPS = [(g, 128) for g in range(11)] + [(11, 36)]

SIG = mybir.ActivationFunctionType.Sigmoid

last_exec_time_ns = None
_cached = None


def _knob(name, default):
    return int(os.environ.get(name, default))


def _host_grid():
    # grid[p, s, a*2+c]: slot s = g*4+t covers output row hw = g*512 + 4p + t
    p = np.arange(128, dtype=np.int64)[:, None]
    s = np.arange(48, dtype=np.int64)[None, :]
    hw = (s // 4) * 512 + 4 * p + (s % 4)
    hw = np.minimum(hw, HW - 1)  # pad slots past the end; never read
    gx = (hw % W).astype(np.float64)
    gy = (hw // W).astype(np.float64)
    g = np.empty((128, 48, 2), dtype=np.float64)
    g[:, :, 0] = (gx - 0.5 * (XY_SCALE - 1.0)) / W
    g[:, :, 1] = (gy - 0.5 * (XY_SCALE - 1.0)) / H
    return g.astype(np.float32).copy()


def _emit_nav(nc, consts):
    """nav[p, t, 2a+c] = -(2*608)/anchor so (s-1)*nav = (1-s)/av; built
    on-chip with memsets (no DMA traffic, no host input)."""
    nav = consts.tile([128, 4, 6], F32)
    for a in range(3):
        nc.gpsimd.memset(nav[:, :, 2 * a + 0], -(2.0 * 608.0) / ANCHOR_WH[a][0])
        nc.gpsimd.memset(nav[:, :, 2 * a + 1], -(2.0 * 608.0) / ANCHOR_WH[a][1])
    return nav


def _build():
    XBUFS = _knob("K_XBUFS", 2)
    OBUFS = _knob("K_OBUFS", 6)
    TBUFS = _knob("K_TBUFS", 4)
    LCHUNK = _knob("K_LCHUNK", 1536)   # load-dma chunk (hw cols)
    STORE_ENG = _knob("K_STORE_ENG", 1)  # 0=scalar(Act) 1=sync(SP)
    # Loads trigger on Act, stores on SP: two HWDGE queues carry ~half the
    # bytes each (single-queue ring throughput is the risk on real HW;
    # costs only ~0.2us in the cost model).
    LOAD_ENG = _knob("K_LOAD_ENG", 1)    # 0=sync(SP) 1=scalar(Act)

    nc = bacc.Bacc("TRN2", target_bir_lowering=False, debug=False, num_devices=NCORES)
    xt = nc.dram_tensor("x", [IPC, NCH, HW], F32, kind="ExternalInput").ap()
    gt = nc.dram_tensor("grid", [128, 48, 2], F32, kind="ExternalInput").ap()
    ot = nc.dram_tensor("out", [IPC, HW, NCH], F32, kind="ExternalOutput").ap()

    store_dma = {
        0: lambda nc: nc.scalar.dma_start,
        1: lambda nc: nc.sync.dma_start,
        2: lambda nc: nc.gpsimd.dma_start,
    }[STORE_ENG]

    with tile.TileContext(nc) as tc:
        with (
            tc.tile_pool(name="consts", bufs=1) as consts,
            tc.tile_pool(name="xin", bufs=XBUFS) as xin,
            tc.tile_pool(name="psum", bufs=2, space="PSUM") as pp,
            tc.tile_pool(name="outp", bufs=OBUFS) as outp,
            tc.tile_pool(name="tmp", bufs=TBUFS) as tmpp,
        ):
            ident = consts.tile([128, 128], F32)
            make_identity(nc, ident)
            grid = consts.tile([128, 48, 6], F32)
            grid2 = consts.tile([128, 48, 2], F32)
            nav = _emit_nav(nc, consts)

            def emit_group(img, g, P, x0v, x1v, m0):
                ps = pp.tile([128, 4, 512], F32, tag="ps")
                for t in range(4):
                    nc.tensor.transpose(
                        ps[0:P, t, 0:128], x0v[:, m0 : m0 + P, t], ident
                    )
                    nc.tensor.transpose(
                        ps[0:P, t, 128:255],
                        x1v[:, m0 : m0 + P, t],
                        ident[0:127, 0:127],
                    )
                o = outp.tile([128, 4, 255], F32, tag="o")
                t1 = tmpp.tile([128, 4, 6], F32, tag="t1")
                t2 = tmpp.tile([128, 4, 6], F32, tag="t2")

                # one sigmoid over all 1020 cols, straight into the out tile
                nc.scalar.activation(o[0:P], ps[0:P, :, 0:255], SIG)

                ovr = o[0:P].rearrange("p t (a c) -> p t a c", a=3)
                s02 = ovr[:, :, :, 0:2]
                s24 = ovr[:, :, :, 2:4]
                t1v = t1[0:P].rearrange("p t (a c) -> p t a c", a=3)
                t2v = t2[0:P].rearrange("p t (a c) -> p t a c", a=3)
                nvv = nav[0:P].rearrange("p t (a c) -> p t a c", a=3)
                gvv = grid[0:P, 4 * g : 4 * g + 4, :].rearrange(
                    "p t (a c) -> p t a c", a=3
                )

                nc.vector.scalar_tensor_tensor(
                    t1v, s24, -1.0, nvv, AluOpType.add, AluOpType.mult
                )  # (s-1)*(-1/av) = (1-s)/av
                nc.vector.reciprocal(t1[0:P], t1[0:P])  # av/(1-s)
                nc.vector.tensor_mul(t1v, t1v, s24)     # exp(wh)*av = half
                nc.vector.scalar_tensor_tensor(
                    t2v, s02, KSC, gvv, AluOpType.mult, AluOpType.add
                )  # imxy
                nc.vector.tensor_sub(s02, t2v, t1v)
                nc.vector.tensor_add(s24, t2v, t1v)

                # rows g*512 + 4p + t ; per partition one 4080B chunk
                dst = ot[img, g * 512 : g * 512 + 4 * P, :].rearrange(
                    "(p four) c -> p four c", four=4
                )
                store_dma(nc)(dst, o[0:P, :, :])

            # sequential images; whole-image x tiles, chunked load DMAs
            for img in range(IPC):
                x0 = xin.tile([128, HW], F32, tag="x0")
                x1 = xin.tile([128, HW], F32, tag="x1")
                # chunked loads: a monolithic 2.95MB load occupies the
                # DMA engines ~8us and stalls the o-buffer recycle.
                # Last image: split the final chunk so the 144-col tail
                # group's data lands early and its (short) store chain can
                # overlap the full groups' store transfers.
                bounds = list(range(0, HW, LCHUNK)) + [HW]
                if img == IPC - 1:
                    bounds = bounds[:-1] + [5632, HW]
                ldma = nc.scalar.dma_start if LOAD_ENG == 1 else nc.sync.dma_start
                l1dma = nc.gpsimd.dma_start if LOAD_ENG == 2 else ldma
                for a, b in zip(bounds[:-1], bounds[1:]):
                    ldma(x0[:, a:b], xt[img, 0:128, a:b])
                    l1dma(x1[0:127, a:b], xt[img, 128:255, a:b])
                if img == 0:
                    # grid const: DMA only the 2 unique values per slot
                    # (49KB, after the first x chunks own the DMA pipeline),
                    # then expand the anchor axis with strided copies
                    nc.scalar.dma_start(grid2, gt)
                    for a_ in range(3):
                        nc.vector.tensor_copy(
                            grid[:, :, 2 * a_ : 2 * a_ + 2], grid2
                        )
                x0v = x0.rearrange("k (m four) -> k m four", four=4)
                x1v = x1[0:127].rearrange("k (m four) -> k m four", four=4)
                for g, P in GROUPS:
                    emit_group(img, g, P, x0v, x1v, g * 128)
    return nc


def kernel(x):
    global last_exec_time_ns, _cached
    x = np.ascontiguousarray(np.asarray(x, dtype=np.float32))
    assert x.shape == (B, NCH, H, W)
    if _cached is None:
        _cached = _build()
        _cached.finalize()  # Bacc: legalize sync waits + freeze
    nc = _cached
    grid = _host_grid()
    xr = x.reshape(B, NCH, HW)
    in_maps = [
        {"x": np.ascontiguousarray(xr[c * IPC : (c + 1) * IPC]), "grid": grid}
        for c in range(NCORES)
    ]
    res = run_bass_kernel_spmd(nc, in_maps, core_ids=list(range(NCORES)))
    last_exec_time_ns = res.exec_time_ns
    out = np.concatenate(
        [r["out"].reshape(IPC, HW * 3, 85) for r in res.results], axis=0
    )
    return out


# revision 24
# speedup vs baseline: 1.0016x; 1.0016x over previous
"""YOLOv3 detection-layer kernel for Trainium2 (Bass/Tile), 8-core data parallel.

Math (per image, input x [255, 5776] channel-major, f = a*85 + c):
  out_flat[hw, f] = g_f(x[f, hw])   where out_flat is [5776, 255] and the
  full output [17328, 85] is just out_flat reshaped (box = hw*3 + a).
So the kernel is: DMA load (channels on partitions) -> PE transpose-mode
(128x128 tiles, exact routing) into PSUM [hw, 255] -> fused sigmoid +
grid/anchor affine -> contiguous DMA store.

Per anchor a (cols base = 85*a), with s = sigmoid(x) over ALL 255 attrs
(one activation instruction per group; exp comes from the sigmoid identity
exp(z) = s/(1-s), with the anchor scale folded into the reciprocal):
  t1 = (s_wh - 1) * (-1/av)   # = (1-s)/av,  av = anchor_wh/(2*608)
  t1 = 1/t1                   # = av/(1-s)
  t1 = t1 * s_wh              # = exp(wh)*av = half
  t2 = s_xy * (1.05/76) + (g-0.025)/76   # = imxy
  out[0:2] = t2 - t1 ; out[2:4] = t2 + t1 ; out[4:85] = s (already there)
"""

import os

import numpy as np

import concourse.bacc as bacc
import concourse.mybir as mybir
import concourse.tile as tile
from concourse.alu_op_type import AluOpType
from concourse.bass_utils import run_bass_kernel_spmd
from concourse.masks import make_identity

F32 = mybir.dt.float32

B = 32            # batch
NCH = 255         # channels = 3 anchors * 85 attrs
H = W = 76
HW = H * W        # 5776
NCORES = 8
IPC = B // NCORES  # images per core
XY_SCALE = 1.05
KSC = XY_SCALE / W
ANCHOR_WH = [(10.0, 13.0), (16.0, 30.0), (33.0, 23.0)]

# Each group owns 4 PSUM banks and covers 512 (tail: 144) consecutive output
# rows. Within a group, PSUM partition p of bank t holds output row
# base + 4p + t, so each partition stores ONE contiguous 4080B DRAM chunk
# (4 adjacent 1020B rows) -> 128 descriptors per store instead of 512.
# (group_index, partitions) ; group 11 is the 144-row tail (36 partitions).
GROUPS = [(g, 128) for g in range(11)] + [(11, 36)]

SIG = mybir.ActivationFunctionType.Sigmoid

last_exec_time_ns = None
_cached = None


def _knob(name, default):
    return int(os.environ.get(name, default))


def _host_grid():
    # grid[p, s, a*2+c]: slot s = g*4+t covers output row hw = g*512 + 4p + t
    p = np.arange(128, dtype=np.int64)[:, None]
    s = np.arange(48, dtype=np.int64)[None, :]
    hw = (s // 4) * 512 + 4 * p + (s % 4)
    hw = np.minimum(hw, HW - 1)  # pad slots past the end; never read
    gx = (hw % W).astype(np.float64)
    gy = (hw // W).astype(np.float64)
    g = np.empty((128, 48, 2), dtype=np.float64)
    g[:, :, 0] = (gx - 0.5 * (XY_SCALE - 1.0)) / W
    g[:, :, 1] = (gy - 0.5 * (XY_SCALE - 1.0)) / H
    return g.astype(np.float32).copy()


def _emit_nav(nc, consts):
    """nav[p, t, 2a+c] = -(2*608)/anchor so (s-1)*nav = (1-s)/av; built
    on-chip with memsets (no DMA traffic, no host input)."""
    nav = consts.tile([128, 4, 6], F32)
    for a in range(3):
        nc.gpsimd.memset(nav[:, :, 2 * a + 0], -(2.0 * 608.0) / ANCHOR_WH[a][0])
        nc.gpsimd.memset(nav[:, :, 2 * a + 1], -(2.0 * 608.0) / ANCHOR_WH[a][1])
    return nav


def _build():
    XBUFS = _knob("K_XBUFS", 2)
    OBUFS = _knob("K_OBUFS", 6)
    TBUFS = _knob("K_TBUFS", 4)
    LCHUNK = _knob("K_LCHUNK", 1536)   # load-dma chunk (hw cols)
    STORE_ENG = _knob("K_STORE_ENG", 1)  # 0=scalar(Act) 1=sync(SP)
    # Loads trigger on Act, stores on SP: two HWDGE queues carry ~half the
    # bytes each (single-queue ring throughput is the risk on real HW;
    # costs only ~0.2us in the cost model).
    LOAD_ENG = _knob("K_LOAD_ENG", 1)    # 0=sync(SP) 1=scalar(Act)

    nc = bacc.Bacc("TRN2", target_bir_lowering=False, debug=False, num_devices=NCORES)
    xt = nc.dram_tensor("x", [IPC, NCH, HW], F32, kind="ExternalInput").ap()
    gt = nc.dram_tensor("grid", [128, 48, 2], F32, kind="ExternalInput").ap()
    ot = nc.dram_tensor("out", [IPC, HW, NCH], F32, kind="ExternalOutput").ap()

    store_dma = {
        0: lambda nc: nc.scalar.dma_start,
        1: lambda nc: nc.sync.dma_start,
        2: lambda nc: nc.gpsimd.dma_start,
    }[STORE_ENG]

    with tile.TileContext(nc) as tc:
        with (
            tc.tile_pool(name="consts", bufs=1) as consts,
            tc.tile_pool(name="xin", bufs=XBUFS) as xin,
            tc.tile_pool(name="psum", bufs=2, space="PSUM") as pp,
            tc.tile_pool(name="outp", bufs=OBUFS) as outp,
            tc.tile_pool(name="tmp", bufs=TBUFS) as tmpp,
        ):
            ident = consts.tile([128, 128], F32)
            make_identity(nc, ident)
            grid = consts.tile([128, 48, 6], F32)
            grid2 = consts.tile([128, 48, 2], F32)
            nav = _emit_nav(nc, consts)

            def emit_group(img, g, P, x0v, x1v, m0):
                ps = pp.tile([128, 4, 512], F32, tag="ps")
                for t in range(4):
                    nc.tensor.transpose(
                        ps[0:P, t, 0:128], x0v[:, m0 : m0 + P, t], ident
                    )
                    nc.tensor.transpose(
                        ps[0:P, t, 128:255],
                        x1v[:, m0 : m0 + P, t],
                        ident[0:127, 0:127],
                    )
                o = outp.tile([128, 4, 255], F32, tag="o")
                t1 = tmpp.tile([128, 4, 6], F32, tag="t1")
                t2 = tmpp.tile([128, 4, 6], F32, tag="t2")

                # one sigmoid over all 1020 cols, straight into the out tile
                nc.scalar.activation(o[0:P], ps[0:P, :, 0:255], SIG)

                ovr = o[0:P].rearrange("p t (a c) -> p t a c", a=3)
                s02 = ovr[:, :, :, 0:2]
                s24 = ovr[:, :, :, 2:4]
                t1v = t1[0:P].rearrange("p t (a c) -> p t a c", a=3)
                t2v = t2[0:P].rearrange("p t (a c) -> p t a c", a=3)
                nvv = nav[0:P].rearrange("p t (a c) -> p t a c", a=3)
                gvv = grid[0:P, 4 * g : 4 * g + 4, :].rearrange(
                    "p t (a c) -> p t a c", a=3
                )

                nc.vector.scalar_tensor_tensor(
                    t1v, s24, -1.0, nvv, AluOpType.add, AluOpType.mult
                )  # (s-1)*(-1/av) = (1-s)/av
                nc.vector.reciprocal(t1[0:P], t1[0:P])  # av/(1-s)
                nc.vector.tensor_mul(t1v, t1v, s24)     # exp(wh)*av = half
                nc.vector.scalar_tensor_tensor(
                    t2v, s02, KSC, gvv, AluOpType.mult, AluOpType.add
                )  # imxy
                nc.vector.tensor_sub(s02, t2v, t1v)
                nc.vector.tensor_add(s24, t2v, t1v)

                # rows g*512 + 4p + t ; per partition one 4080B chunk
                dst = ot[img, g * 512 : g * 512 + 4 * P, :].rearrange(
                    "(p four) c -> p four c", four=4
                )
                store_dma(nc)(dst, o[0:P, :, :])

            # sequential images; whole-image x tiles, chunked load DMAs
            for img in range(IPC):
                x0 = xin.tile([128, HW], F32, tag="x0")
                x1 = xin.tile([128, HW], F32, tag="x1")
                # chunked loads: a monolithic 2.95MB load occupies the
                # DMA engines ~8us and stalls the o-buffer recycle.
                # Last image: split the final chunk so the 144-col tail
                # group's data lands early and its (short) store chain can
                # overlap the full groups' store transfers.
                bounds = list(range(0, HW, LCHUNK)) + [HW]
                if img == IPC - 1:
                    bounds = bounds[:-1] + [5632, HW]
                ldma = nc.scalar.dma_start if LOAD_ENG == 1 else nc.sync.dma_start
                l1dma = nc.gpsimd.dma_start if LOAD_ENG == 2 else ldma
                for a, b in zip(bounds[:-1], bounds[1:]):
                    # very first chunk issues on SP: its queue is store-only
                    # (idle until ~8us) and has the shorter issue pipeline,
                    # so the first transfer starts ~216ns earlier
                    fdma = nc.sync.dma_start if (img == 0 and a == 0) else ldma
                    fdma(x0[:, a:b], xt[img, 0:128, a:b])
                    fdma(x1[0:127, a:b], xt[img, 128:255, a:b])
                if img == 0:
                    # grid const: DMA only the 2 unique values per slot
                    # (49KB, after the first x chunks own the DMA pipeline),
                    # then expand the anchor axis with strided copies
                    nc.scalar.dma_start(grid2, gt)
                    for a_ in range(3):
                        nc.vector.tensor_copy(
                            grid[:, :, 2 * a_ : 2 * a_ + 2], grid2
                        )
                x0v = x0.rearrange("k (m four) -> k m four", four=4)
                x1v = x1[0:127].rearrange("k (m four) -> k m four", four=4)
                for g, P in GROUPS:
                    emit_group(img, g, P, x0v, x1v, g * 128)
    return nc


def kernel(x):
    global last_exec_time_ns, _cached
    x = np.ascontiguousarray(np.asarray(x, dtype=np.float32))
    assert x.shape == (B, NCH, H, W)
    if _cached is None:
        _cached = _build()
        _cached.finalize()  # Bacc: legalize sync waits + freeze
    nc = _cached
    grid = _host_grid()
    xr = x.reshape(B, NCH, HW)
    in_maps = [
        {"x": np.ascontiguousarray(xr[c * IPC : (c + 1) * IPC]), "grid": grid}
        for c in range(NCORES)
    ]
    res = run_bass_kernel_spmd(nc, in_maps, core_ids=list(range(NCORES)))
    last_exec_time_ns = res.exec_time_ns
    out = np.concatenate(
        [r["out"].reshape(IPC, HW * 3, 85) for r in res.results], axis=0
    )
    return out


# revision 25
# speedup vs baseline: 1.0016x; 1.0000x over previous
"""YOLOv3 detection-layer kernel for Trainium2 (Bass/Tile), 8-core data parallel.

Math (per image, input x [255, 5776] channel-major, f = a*85 + c):
  out_flat[hw, f] = g_f(x[f, hw])   where out_flat is [5776, 255] and the
  full output [17328, 85] is just out_flat reshaped (box = hw*3 + a).
So the kernel is: DMA load (channels on partitions) -> PE transpose-mode
(128x128 tiles, exact routing) into PSUM [hw, 255] -> fused sigmoid +
grid/anchor affine -> contiguous DMA store.

Per anchor a (cols base = 85*a), with s = sigmoid(x) over ALL 255 attrs
(one activation instruction per group; exp comes from the sigmoid identity
exp(z) = s/(1-s), with the anchor scale folded into the reciprocal):
  t1 = (s_wh - 1) * (-1/av)   # = (1-s)/av,  av = anchor_wh/(2*608)
  t1 = 1/t1                   # = av/(1-s)
  t1 = t1 * s_wh              # = exp(wh)*av = half
  t2 = s_xy * (1.05/76) + (g-0.025)/76   # = imxy
  out[0:2] = t2 - t1 ; out[2:4] = t2 + t1 ; out[4:85] = s (already there)
"""

import os

import numpy as np

import concourse.bacc as bacc
import concourse.mybir as mybir
import concourse.tile as tile
from concourse.alu_op_type import AluOpType
from concourse.bass_utils import run_bass_kernel_spmd
from concourse.masks import make_identity

F32 = mybir.dt.float32

B = 32            # batch
NCH = 255         # channels = 3 anchors * 85 attrs
H = W = 76
HW = H * W        # 5776
NCORES = 8
IPC = B // NCORES  # images per core
XY_SCALE = 1.05
KSC = XY_SCALE / W
ANCHOR_WH = [(10.0, 13.0), (16.0, 30.0), (33.0, 23.0)]

# Each group owns 4 PSUM banks and covers 512 (tail: 144) consecutive output
# rows. Within a group, PSUM partition p of bank t holds output row
# base + 4p + t, so each partition stores ONE contiguous 4080B DRAM chunk
# (4 adjacent 1020B rows) -> 128 descriptors per store instead of 512.
# (group_index, partitions) ; group 11 is the 144-row tail (36 partitions).
GROUPS = [(g, 128) for g in range(11)] + [(11, 36)]

SIG = mybir.ActivationFunctionType.Sigmoid

last_exec_time_ns = None
_cached = None


def _knob(name, default):
    return int(os.environ.get(name, default))


def _host_grid():
    # grid[p, s, a*2+c]: slot s = g*4+t covers output row hw = g*512 + 4p + t
    p = np.arange(128, dtype=np.int64)[:, None]
    s = np.arange(48, dtype=np.int64)[None, :]
    hw = (s // 4) * 512 + 4 * p + (s % 4)
    hw = np.minimum(hw, HW - 1)  # pad slots past the end; never read
    gx = (hw % W).astype(np.float64)
    gy = (hw // W).astype(np.float64)
    g = np.empty((128, 48, 2), dtype=np.float64)
    g[:, :, 0] = (gx - 0.5 * (XY_SCALE - 1.0)) / W
    g[:, :, 1] = (gy - 0.5 * (XY_SCALE - 1.0)) / H
    return g.astype(np.float32).copy()


def _emit_nav(nc, consts):
    """nav[p, t, 2a+c] = -(2*608)/anchor so (s-1)*nav = (1-s)/av; built
    on-chip with memsets (no DMA traffic, no host input)."""
    nav = consts.tile([128, 4, 6], F32)
    for a in range(3):
        nc.gpsimd.memset(nav[:, :, 2 * a + 0], -(2.0 * 608.0) / ANCHOR_WH[a][0])
        nc.gpsimd.memset(nav[:, :, 2 * a + 1], -(2.0 * 608.0) / ANCHOR_WH[a][1])
    return nav


def _build():
    XBUFS = _knob("K_XBUFS", 2)
    OBUFS = _knob("K_OBUFS", 6)
    TBUFS = _knob("K_TBUFS", 4)
    # 2888 (half image) is sim-equal to smaller chunks but halves the load
    # descriptor count (128 x 11.5KB per DMA) -> less ring overhead on HW
    LCHUNK = _knob("K_LCHUNK", 2888)   # load-dma chunk (hw cols)
    STORE_ENG = _knob("K_STORE_ENG", 1)  # 0=scalar(Act) 1=sync(SP)
    # Loads trigger on Act, stores on SP: two HWDGE queues carry ~half the
    # bytes each (single-queue ring throughput is the risk on real HW;
    # costs only ~0.2us in the cost model).
    LOAD_ENG = _knob("K_LOAD_ENG", 1)    # 0=sync(SP) 1=scalar(Act)

    nc = bacc.Bacc("TRN2", target_bir_lowering=False, debug=False, num_devices=NCORES)
    xt = nc.dram_tensor("x", [IPC, NCH, HW], F32, kind="ExternalInput").ap()
    gt = nc.dram_tensor("grid", [128, 48, 2], F32, kind="ExternalInput").ap()
    ot = nc.dram_tensor("out", [IPC, HW, NCH], F32, kind="ExternalOutput").ap()

    store_dma = {
        0: lambda nc: nc.scalar.dma_start,
        1: lambda nc: nc.sync.dma_start,
        2: lambda nc: nc.gpsimd.dma_start,
    }[STORE_ENG]

    with tile.TileContext(nc) as tc:
        with (
            tc.tile_pool(name="consts", bufs=1) as consts,
            tc.tile_pool(name="xin", bufs=XBUFS) as xin,
            tc.tile_pool(name="psum", bufs=2, space="PSUM") as pp,
            tc.tile_pool(name="outp", bufs=OBUFS) as outp,
            tc.tile_pool(name="tmp", bufs=TBUFS) as tmpp,
        ):
            ident = consts.tile([128, 128], F32)
            make_identity(nc, ident)
            grid = consts.tile([128, 48, 6], F32)
            grid2 = consts.tile([128, 48, 2], F32)
            nav = _emit_nav(nc, consts)

            def emit_group(img, g, P, x0v, x1v, m0):
                ps = pp.tile([128, 4, 512], F32, tag="ps")
                for t in range(4):
                    nc.tensor.transpose(
                        ps[0:P, t, 0:128], x0v[:, m0 : m0 + P, t], ident
                    )
                    nc.tensor.transpose(
                        ps[0:P, t, 128:255],
                        x1v[:, m0 : m0 + P, t],
                        ident[0:127, 0:127],
                    )
                o = outp.tile([128, 4, 255], F32, tag="o")
                t1 = tmpp.tile([128, 4, 6], F32, tag="t1")
                t2 = tmpp.tile([128, 4, 6], F32, tag="t2")

                # one sigmoid over all 1020 cols, straight into the out tile
                nc.scalar.activation(o[0:P], ps[0:P, :, 0:255], SIG)

                ovr = o[0:P].rearrange("p t (a c) -> p t a c", a=3)
                s02 = ovr[:, :, :, 0:2]
                s24 = ovr[:, :, :, 2:4]
                t1v = t1[0:P].rearrange("p t (a c) -> p t a c", a=3)
                t2v = t2[0:P].rearrange("p t (a c) -> p t a c", a=3)
                nvv = nav[0:P].rearrange("p t (a c) -> p t a c", a=3)
                gvv = grid[0:P, 4 * g : 4 * g + 4, :].rearrange(
                    "p t (a c) -> p t a c", a=3
                )

                nc.vector.scalar_tensor_tensor(
                    t1v, s24, -1.0, nvv, AluOpType.add, AluOpType.mult
                )  # (s-1)*(-1/av) = (1-s)/av
                nc.vector.reciprocal(t1[0:P], t1[0:P])  # av/(1-s)
                nc.vector.tensor_mul(t1v, t1v, s24)     # exp(wh)*av = half
                nc.vector.scalar_tensor_tensor(
                    t2v, s02, KSC, gvv, AluOpType.mult, AluOpType.add
                )  # imxy
                nc.vector.tensor_sub(s02, t2v, t1v)
                nc.vector.tensor_add(s24, t2v, t1v)

                # rows g*512 + 4p + t ; per partition one 4080B chunk
                dst = ot[img, g * 512 : g * 512 + 4 * P, :].rearrange(
                    "(p four) c -> p four c", four=4
                )
                store_dma(nc)(dst, o[0:P, :, :])

            # sequential images; whole-image x tiles, chunked load DMAs
            for img in range(IPC):
                x0 = xin.tile([128, HW], F32, tag="x0")
                x1 = xin.tile([128, HW], F32, tag="x1")
                # chunked loads: a monolithic 2.95MB load occupies the
                # DMA engines ~8us and stalls the o-buffer recycle.
                # Last image: split the final chunk so the 144-col tail
                # group's data lands early and its (short) store chain can
                # overlap the full groups' store transfers.
                bounds = list(range(0, HW, LCHUNK)) + [HW]
                if img == IPC - 1:
                    bounds = bounds[:-1] + [5632, HW]
                ldma = nc.scalar.dma_start if LOAD_ENG == 1 else nc.sync.dma_start
                l1dma = nc.gpsimd.dma_start if LOAD_ENG == 2 else ldma
                for a, b in zip(bounds[:-1], bounds[1:]):
                    # very first chunk issues on SP: its queue is store-only
                    # (idle until ~8us) and has the shorter issue pipeline,
                    # so the first transfer starts ~216ns earlier
                    fdma = nc.sync.dma_start if (img == 0 and a == 0) else ldma
                    fdma(x0[:, a:b], xt[img, 0:128, a:b])
                    fdma(x1[0:127, a:b], xt[img, 128:255, a:b])
                if img == 0:
                    # grid const: DMA only the 2 unique values per slot
                    # (49KB, after the first x chunks own the DMA pipeline),
                    # then expand the anchor axis with strided copies
                    nc.scalar.dma_start(grid2, gt)
                    for a_ in range(3):
                        nc.vector.tensor_copy(
                            grid[:, :, 2 * a_ : 2 * a_ + 2], grid2
                        )
                x0v = x0.rearrange("k (m four) -> k m four", four=4)
                x1v = x1[0:127].rearrange("k (m four) -> k m four", four=4)
                for g, P in GROUPS:
                    emit_group(img, g, P, x0v, x1v, g * 128)
    return nc


def kernel(x):
    global last_exec_time_ns, _cached
    x = np.ascontiguousarray(np.asarray(x, dtype=np.float32))
    assert x.shape == (B, NCH, H, W)
    if _cached is None:
        _cached = _build()
        _cached.finalize()  # Bacc: legalize sync waits + freeze
    nc = _cached
    grid = _host_grid()
    xr = x.reshape(B, NCH, HW)
    in_maps = [
        {"x": np.ascontiguousarray(xr[c * IPC : (c + 1) * IPC]), "grid": grid}
        for c in range(NCORES)
    ]
    res = run_bass_kernel_spmd(nc, in_maps, core_ids=list(range(NCORES)))
    last_exec_time_ns = res.exec_time_ns
    out = np.concatenate(
        [r["out"].reshape(IPC, HW * 3, 85) for r in res.results], axis=0
    )
    return out
